# revision 1
# baseline (speedup 1.0000x reference)
"""Trainium2 Bass kernel for NnqlmCnnBasedLstm.

Math (per batch item, per input sequence q/a):
  xe = embed[idx]                      (L, D)       D = 128
  dens_t = outer(xe_t, xe_t)/(|xe_t|^2 + 1e-4)     (D, D), symmetric
  2-layer ConvLSTM over L=40 steps; each gate g:
    pre_g = conv2d([xt; h], W_g, stride=(2,1), pad=(1,1)) + b_g  on (2D, D) -> (D, D)
  c = sig(f)*c + sig(i)*tanh(cc); h = sig(o)*tanh(c)
  out = max_t h2_t  -> flatten -> concat(q,a) -> linear(2) -> log_softmax

Device strategy (8 cores, data parallel over B=32 -> 4 items/core, each with a
q-chain and an a-chain = 8 chains/core):
  * State kept TRANSPOSED: tiles are (w partitions, j free).  The density is
    symmetric so layer-1 inputs need no transpose.
  * conv: out_T[w, j] = sum_{dh,dw} W[dh,dw] * inp_T[w-1+dw, 2j-1+dh].
    For each dh this is a 3-diagonal Toeplitz band matrix (over w) applied via
    the TensorEngine, with the (2j-1+dh) selection expressed as a stride-2
    free-axis access pattern on the moving operand.  4 gates x 4 dh matmuls
    accumulate in PSUM; all 8 chains batched in the moving free dim.
  * sigmoid/tanh (+conv bias) on ScalarE reading PSUM; cell updates on VectorE;
    densities as rank-1 (K=1) outer-product matmuls on the TensorEngine.
  * Embedding gather, final linear + log_softmax on host (tiny).
"""

import os
import sys

import numpy as np

for _p in ("/opt/trn_rl_repo", "/root/.axon_site/_ro/trn_rl_repo"):
    if os.path.isdir(_p) and _p not in sys.path:
        sys.path.insert(0, _p)

B, L, D, V, NL = 32, 40, 128, 32000, 2
NCORES = 8
CH = 8            # chains per core: 4 batch items x {q, a}
SEG = 2 * D + 2   # per-chain column span in the input tile: [0]=0, [1..128]=x, [129..256]=h, [257]=0
NF = CH * SEG
NV = L * CH       # 320 embedding vectors per core
NVP = 384         # padded to a multiple of 128

_CACHE = {}


def _build_nc(L=L):
    import concourse.bass as bass
    import concourse.bacc as bacc
    import concourse.mybir as mybir
    from concourse import tile

    f32 = mybir.dt.float32
    AF = mybir.ActivationFunctionType
    ALU = mybir.AluOpType

    nc = bacc.Bacc(None, target_bir_lowering=False)

    xey_d = nc.dram_tensor("xey", (L, 1, CH * D), f32, kind="ExternalInput")
    st_d = nc.dram_tensor("st", (NL * 4 * 4, D, D), f32, kind="ExternalInput")
    bias_d = nc.dram_tensor("bias", (D, NL * 4), f32, kind="ExternalInput")
    out_d = nc.dram_tensor("mp_out", (D, CH * D), f32, kind="ExternalOutput")

    with tile.TileContext(nc) as tc:
        with (
            tc.tile_pool(name="const", bufs=1) as constp,
            tc.tile_pool(name="state", bufs=1) as statep,
            tc.tile_pool(name="inp", bufs=2) as inpp,
            tc.tile_pool(name="gate", bufs=2) as gatep,
            tc.tile_pool(name="psum", bufs=1, space="PSUM") as psump,
        ):
            # ---- constants ----
            stT = constp.tile([D, NL * 4 * 4 * D], f32, tag="stT")
            for i in range(NL * 4 * 4):
                nc.sync.dma_start(stT[:, i * D:(i + 1) * D], st_d[i])

            bias = constp.tile([D, NL * 4], f32, tag="bias")
            nc.sync.dma_start(bias[:], bias_d[:])

            # ---- persistent state ----
            c_l = [statep.tile([D, CH * D], f32, tag=f"c{l}", name=f"c{l}") for l in range(NL)]
            mp = statep.tile([D, CH * D], f32, tag="mp")
            for l in range(NL):
                nc.vector.memset(c_l[l][:], 0.0)
            nc.vector.memset(mp[:], -1e30)

            def seg3(t):  # (p, s, c) view of an input tile
                return t[:].rearrange("p (s c) -> p s c", s=CH)

            def seg4(t):  # (p, s, c2, two) parity view for stride-2 j access
                return t[:].rearrange("p (s c two) -> p s c two", s=CH, two=2)

            def new_inp(tag):
                t = inpp.tile([D, NF], f32, tag=tag, name=tag)
                # zero the pad columns (0 and 257 of each chain segment)
                v = t[:].rearrange("p (s c) -> p s c", s=CH)
                nc.gpsimd.memset(v[:, :, 0:1], 0.0)
                nc.gpsimd.memset(v[:, :, SEG - 1:SEG], 0.0)
                return t

            def outers(t_next, dst_tile):
                """Rank-1 matmuls: densities for step t_next -> x-part of dst_tile."""
                stage = gatep.tile([1, CH * D], f32, tag="xstage", name="xstage")
                nc.sync.dma_start(stage[:], xey_d[t_next])
                po = psump.tile([D, CH * D], f32, tag="pf", name="po")
                for s in range(CH):
                    vec = stage[0:1, s * D:(s + 1) * D]
                    nc.tensor.matmul(
                        po[:, s * D:(s + 1) * D],
                        vec, vec,
                        start=True, stop=True,
                    )
                v3 = seg3(dst_tile)
                for hf in range(2):
                    nc.scalar.activation(v3[:, hf * 4:(hf + 1) * 4, 1:1 + D],
                                         po[:, hf * 512:(hf + 1) * 512], AF.Copy)

            cur = [None, None]
            cur[0] = new_inp("inp0")
            cur[1] = new_inp("inp1")
            nc.gpsimd.memset(seg3(cur[0])[:, :, 129:129 + D], 0.0)   # h1_{-1} = 0
            nc.gpsimd.memset(seg3(cur[1])[:, :, 129:129 + D], 0.0)   # h2_{-1} = 0
            outers(0, cur[0])

            GTAG = ["pf", "pi", "po", "pc"]
            for t in range(L):
                nxt = [None, None]
                nxt[0] = new_inp("inp0") if t + 1 < L else None
                nxt[1] = new_inp("inp1") if t + 1 < L else None

                for l in range(NL):
                    inp = cur[l]
                    i4 = seg4(inp)
                    # --- gate pre-activations: 4 gates x 4 dh band matmuls ---
                    ps = [psump.tile([D, CH * D], f32, tag=GTAG[g], name=GTAG[g]) for g in range(4)]
                    for g in range(4):
                        for half in range(2):
                            for dh in range(4):
                                idx = (l * 4 + g) * 4 + dh
                                rhs = i4[:, half * 4:(half + 1) * 4,
                                         dh // 2: dh // 2 + D, dh % 2]
                                nc.tensor.matmul(
                                    ps[g][:, half * 512:(half + 1) * 512],
                                    stT[:, idx * D:(idx + 1) * D],
                                    rhs,
                                    start=(dh == 0), stop=(dh == 3),
                                )
                    # --- activations (bias folded in) ---
                    fg = gatep.tile([D, CH * D], f32, tag="fg")
                    ig = gatep.tile([D, CH * D], f32, tag="ig")
                    og = gatep.tile([D, CH * D], f32, tag="og")
                    cs = gatep.tile([D, CH * D], f32, tag="cs")
                    for g, dst in enumerate((fg, ig, og)):
                        nc.scalar.activation(dst[:], ps[g][:], AF.Sigmoid,
                                             bias=bias[:, l * 4 + g: l * 4 + g + 1])
                    nc.scalar.activation(cs[:], ps[3][:], AF.Tanh,
                                         bias=bias[:, l * 4 + 3: l * 4 + 4])
                    # --- cell update ---
                    t1 = gatep.tile([D, CH * D], f32, tag="t1")
                    t2 = gatep.tile([D, CH * D], f32, tag="t2")
                    nc.vector.tensor_mul(t1[:], fg[:], c_l[l][:])
                    nc.vector.tensor_mul(t2[:], ig[:], cs[:])
                    nc.vector.tensor_add(c_l[l][:], t1[:], t2[:])
                    th = gatep.tile([D, CH * D], f32, tag="th")
                    nc.scalar.activation(th[:], c_l[l][:], AF.Tanh)
                    # --- h = og * tanh(c): route to consumers ---
                    if l == 0:
                        # h1_t -> x-part of layer-2 input (this step)
                        nc.vector.tensor_mul(seg3(cur[1])[:, :, 1:1 + D], og[:], th[:])
                        if nxt[0] is not None:
                            # copy h1_t -> h-part of next layer-1 input
                            nc.gpsimd.tensor_copy(
                                seg3(nxt[0])[:, :, 129:129 + D],
                                seg3(cur[1])[:, :, 1:1 + D],
                            )
                    else:
                        if nxt[1] is not None:
                            h2dst = seg3(nxt[1])[:, :, 129:129 + D]
                            nc.vector.tensor_mul(h2dst, og[:], th[:])
                            nc.vector.tensor_tensor(mp[:], mp[:], h2dst, op=ALU.max)
                        else:
                            h2 = gatep.tile([D, CH * D], f32, tag="h2last")
                            nc.vector.tensor_mul(h2[:], og[:], th[:])
                            nc.vector.tensor_tensor(mp[:], mp[:], h2[:], op=ALU.max)

                if nxt[0] is not None:
                    outers(t + 1, nxt[0])
                cur = nxt

            nc.sync.dma_start(out_d[:], mp[:])

    nc.compile()
    return nc


def _prep_core_inputs(xe_y, st, bias_arr, core):
    """xe_y: (B, 2, L, D) sqrt-normalized embeddings (axis1: 0=q, 1=a)."""
    sl = slice(4 * core, 4 * core + 4)
    # chains: s=0..3 -> q items, s=4..7 -> a items
    ch = np.concatenate([xe_y[sl, 0], xe_y[sl, 1]], axis=0)    # (8, L, D)
    xey = np.ascontiguousarray(ch.transpose(1, 0, 2)).reshape(L, 1, CH * D)
    return {"xey": xey, "st": st, "bias": bias_arr}


def kernel(q, a, embed, conv_w, conv_b, lin_w, lin_b):
    from concourse import bass_utils

    q = np.asarray(q); a = np.asarray(a)
    embed = np.asarray(embed, np.float32)
    conv_w = np.asarray(conv_w, np.float32)
    conv_b = np.asarray(conv_b, np.float32)
    lin_w = np.asarray(lin_w, np.float32)
    lin_b = np.asarray(lin_b, np.float32)

    # host: embedding gather + density normalization factors
    idx = np.stack([q, a], axis=1).astype(np.int64)            # (B, 2, L)
    xe = embed[idx].astype(np.float64)                         # (B, 2, L, D)
    dot = np.sum(xe * xe, axis=-1, keepdims=True) + 1e-4
    xe_y = (xe / np.sqrt(dot)).astype(np.float32)

    # host: Toeplitz band stationaries  lhsT[(l,g,dh)] = B^T,
    # B[w, w'] = W[dh, w'-w+1]  (3 diagonals)
    st = np.zeros((NL * 4 * 4, D, D), np.float32)
    for l in range(NL):
        for g in range(4):
            W = conv_w[l, g, 0, 0]                             # (4, 3)
            for dh in range(4):
                Bm = sum(W[dh, dw] * np.eye(D, k=dw - 1) for dw in range(3))
                st[(l * 4 + g) * 4 + dh] = Bm.T.astype(np.float32)
    bias_arr = np.tile(conv_b.reshape(1, -1), (D, 1)).astype(np.float32)

    if "nc" not in _CACHE:
        _CACHE["nc"] = _build_nc()
    nc = _CACHE["nc"]

    in_maps = [_prep_core_inputs(xe_y, st, bias_arr, i) for i in range(NCORES)]
    _CACHE["in_maps"] = in_maps
    res = bass_utils.run_bass_kernel_spmd(nc, in_maps, core_ids=list(range(NCORES)))

    # host: unshard + final linear + log_softmax
    q_p = np.zeros((B, D * D), np.float32)
    a_p = np.zeros((B, D * D), np.float32)
    for i in range(NCORES):
        out = res.results[i]["mp_out"]                         # (D w, CH*D)
        for s in range(CH):
            mp_T = out[:, s * D:(s + 1) * D]                   # (w, j)
            flat = np.ascontiguousarray(mp_T.T).reshape(-1)    # j-major
            if s < 4:
                q_p[4 * i + s] = flat
            else:
                a_p[4 * i + s - 4] = flat
    qa = np.concatenate([q_p, a_p], axis=1)
    score = qa @ lin_w.T + lin_b
    m = score.max(axis=1, keepdims=True)
    ls = score - m
    lse = np.log(np.exp(ls).sum(axis=1, keepdims=True))
    return (ls - lse).astype(np.float32)



# revision 2
# speedup vs baseline: 3.4989x; 3.4989x over previous
"""Trainium2 Bass kernel for NnqlmCnnBasedLstm.

Math (per batch item, per input sequence q/a):
  xe = embed[idx]                      (L, D)       D = 128
  dens_t = outer(xe_t, xe_t)/(|xe_t|^2 + 1e-4)     (D, D), symmetric
  2-layer ConvLSTM over L=40 steps; each gate g:
    pre_g = conv2d([xt; h], W_g, stride=(2,1), pad=(1,1)) + b_g  on (2D, D) -> (D, D)
  c = sig(f)*c + sig(i)*tanh(cc); h = sig(o)*tanh(c)
  out = max_t h2_t  -> flatten -> concat(q,a) -> linear(2) -> log_softmax

Device strategy (8 cores, data parallel over B=32 -> 4 items/core, each with a
q-chain and an a-chain = 8 chains/core):
  * State kept TRANSPOSED: tiles are (w partitions, j free).  The density is
    symmetric so layer-1 inputs need no transpose.
  * conv: out_T[w, j] = sum_{dh,dw} W[dh,dw] * inp_T[w-1+dw, 2j-1+dh].
    For each dh this is a 3-diagonal Toeplitz band matrix (over w) applied via
    the TensorEngine, with the (2j-1+dh) selection expressed as a stride-2
    free-axis access pattern on the moving operand.  4 gates x 4 dh matmuls
    accumulate in PSUM; all 8 chains batched in the moving free dim.
  * sigmoid/tanh (+conv bias) on ScalarE reading PSUM; cell updates on VectorE;
    densities as rank-1 (K=1) outer-product matmuls on the TensorEngine.
  * Embedding gather, final linear + log_softmax on host (tiny).
"""

import os
import sys

import numpy as np

for _p in ("/opt/trn_rl_repo", "/root/.axon_site/_ro/trn_rl_repo"):
    if os.path.isdir(_p) and _p not in sys.path:
        sys.path.insert(0, _p)

B, L, D, V, NL = 32, 40, 128, 32000, 2
NCORES = 8
CH = 8            # chains per core: 4 batch items x {q, a}
SEG = 2 * D + 2   # per-chain column span in the input tile: [0]=0, [1..128]=x, [129..256]=h, [257]=0
NF = CH * SEG
NV = L * CH       # 320 embedding vectors per core
NVP = 384         # padded to a multiple of 128

_CACHE = {}


def _build_nc(L=L):
    import concourse.bass as bass
    import concourse.bacc as bacc
    import concourse.mybir as mybir
    from concourse import tile

    f32 = mybir.dt.float32
    AF = mybir.ActivationFunctionType
    ALU = mybir.AluOpType

    nc = bacc.Bacc(None, target_bir_lowering=False)

    f16 = mybir.dt.float16
    xey_d = nc.dram_tensor("xey", (L, 1, CH * D), f16, kind="ExternalInput")
    st_d = nc.dram_tensor("st", (NL * 4 * 4, D, D), f16, kind="ExternalInput")
    bias_d = nc.dram_tensor("bias", (D, NL * 4), f32, kind="ExternalInput")
    out_d = nc.dram_tensor("mp_out", (D, CH * D), f16, kind="ExternalOutput")

    with tile.TileContext(nc) as tc:
        with (
            tc.tile_pool(name="const", bufs=1) as constp,
            tc.tile_pool(name="state", bufs=1) as statep,
            tc.tile_pool(name="inp", bufs=2) as inpp,
            tc.tile_pool(name="gate", bufs=2) as gatep,
            tc.tile_pool(name="psum", bufs=1, space="PSUM") as psump,
        ):
            # ---- constants ----
            stT = constp.tile([D, NL * 4 * 4 * D], f16, tag="stT")
            for i in range(NL * 4 * 4):
                nc.sync.dma_start(stT[:, i * D:(i + 1) * D], st_d[i])

            bias = constp.tile([D, NL * 4], f32, tag="bias")
            nc.sync.dma_start(bias[:], bias_d[:])

            # ---- persistent state ----
            c_l = [statep.tile([D, CH * D], f16, tag=f"c{l}", name=f"c{l}") for l in range(NL)]
            mp = statep.tile([D, CH * D], f16, tag="mp")
            for l in range(NL):
                nc.vector.memset(c_l[l][:], 0.0)
            nc.vector.memset(mp[:], -1e30)

            def seg3(t):  # (p, s, c) view of an input tile
                return t[:].rearrange("p (s c) -> p s c", s=CH)

            def seg4(t):  # (p, s, c2, two) parity view for stride-2 j access
                return t[:].rearrange("p (s c two) -> p s c two", s=CH, two=2)

            def new_inp(tag):
                t = inpp.tile([D, NF], f16, tag=tag, name=tag)
                # zero the pad columns (0 and 257 of each chain segment)
                v = t[:].rearrange("p (s c) -> p s c", s=CH)
                nc.gpsimd.memset(v[:, :, 0:1], 0.0)
                nc.gpsimd.memset(v[:, :, SEG - 1:SEG], 0.0)
                return t

            def outers(t_next, dst_tile):
                """Rank-1 matmuls: densities for step t_next -> x-part of dst_tile."""
                stage = gatep.tile([1, CH * D], f16, tag="xstage", name="xstage")
                nc.sync.dma_start(stage[:], xey_d[t_next])
                po = psump.tile([D, CH * D], f32, tag="pf", name="po")
                for s in range(CH):
                    vec = stage[0:1, s * D:(s + 1) * D]
                    nc.tensor.matmul(
                        po[:, s * D:(s + 1) * D],
                        vec, vec,
                        start=True, stop=True,
                    )
                v3 = seg3(dst_tile)
                for hf in range(2):
                    nc.scalar.activation(v3[:, hf * 4:(hf + 1) * 4, 1:1 + D],
                                         po[:, hf * 512:(hf + 1) * 512], AF.Copy)

            cur = [None, None]
            cur[0] = new_inp("inp0")
            cur[1] = new_inp("inp1")
            nc.gpsimd.memset(seg3(cur[0])[:, :, 129:129 + D], 0.0)   # h1_{-1} = 0
            nc.gpsimd.memset(seg3(cur[1])[:, :, 129:129 + D], 0.0)   # h2_{-1} = 0
            outers(0, cur[0])

            GTAG = ["pf", "pi", "po", "pc"]
            for t in range(L):
                nxt = [None, None]
                nxt[0] = new_inp("inp0") if t + 1 < L else None
                nxt[1] = new_inp("inp1") if t + 1 < L else None

                for l in range(NL):
                    inp = cur[l]
                    i4 = seg4(inp)
                    # --- gate pre-activations: 4 gates x 4 dh band matmuls ---
                    ps = [psump.tile([D, CH * D], f32, tag=GTAG[g], name=GTAG[g]) for g in range(4)]
                    for g in range(4):
                        for half in range(2):
                            for dh in range(4):
                                idx = (l * 4 + g) * 4 + dh
                                rhs = i4[:, half * 4:(half + 1) * 4,
                                         dh // 2: dh // 2 + D, dh % 2]
                                nc.tensor.matmul(
                                    ps[g][:, half * 512:(half + 1) * 512],
                                    stT[:, idx * D:(idx + 1) * D],
                                    rhs,
                                    start=(dh == 0), stop=(dh == 3),
                                )
                    # --- activations (bias folded in) ---
                    fg = gatep.tile([D, CH * D], f16, tag="fg")
                    ig = gatep.tile([D, CH * D], f16, tag="ig")
                    og = gatep.tile([D, CH * D], f16, tag="og")
                    cs = gatep.tile([D, CH * D], f16, tag="cs")
                    for g, dst in enumerate((fg, ig, og)):
                        nc.scalar.activation(dst[:], ps[g][:], AF.Sigmoid,
                                             bias=bias[:, l * 4 + g: l * 4 + g + 1])
                    nc.scalar.activation(cs[:], ps[3][:], AF.Tanh,
                                         bias=bias[:, l * 4 + 3: l * 4 + 4])
                    # --- cell update ---
                    t1 = gatep.tile([D, CH * D], f16, tag="t1")
                    t2 = gatep.tile([D, CH * D], f16, tag="t2")
                    nc.vector.tensor_mul(t1[:], fg[:], c_l[l][:])
                    nc.vector.tensor_mul(t2[:], ig[:], cs[:])
                    nc.vector.tensor_add(c_l[l][:], t1[:], t2[:])
                    th = gatep.tile([D, CH * D], f16, tag="th")
                    nc.scalar.activation(th[:], c_l[l][:], AF.Tanh)
                    # --- h = og * tanh(c): route to consumers ---
                    if l == 0:
                        # h1_t -> x-part of layer-2 input (this step)
                        nc.vector.tensor_mul(seg3(cur[1])[:, :, 1:1 + D], og[:], th[:])
                        if nxt[0] is not None:
                            # copy h1_t -> h-part of next layer-1 input
                            nc.gpsimd.tensor_copy(
                                seg3(nxt[0])[:, :, 129:129 + D],
                                seg3(cur[1])[:, :, 1:1 + D],
                            )
                    else:
                        if nxt[1] is not None:
                            h2dst = seg3(nxt[1])[:, :, 129:129 + D]
                            nc.vector.tensor_mul(h2dst, og[:], th[:])
                            nc.vector.tensor_tensor(mp[:], mp[:], h2dst, op=ALU.max)
                        else:
                            h2 = gatep.tile([D, CH * D], f16, tag="h2last")
                            nc.vector.tensor_mul(h2[:], og[:], th[:])
                            nc.vector.tensor_tensor(mp[:], mp[:], h2[:], op=ALU.max)

                if nxt[0] is not None:
                    outers(t + 1, nxt[0])
                cur = nxt

            nc.sync.dma_start(out_d[:], mp[:])

    nc.compile()
    return nc


def _prep_core_inputs(xe_y, st, bias_arr, core):
    """xe_y: (B, 2, L, D) sqrt-normalized embeddings (axis1: 0=q, 1=a)."""
    sl = slice(4 * core, 4 * core + 4)
    # chains: s=0..3 -> q items, s=4..7 -> a items
    ch = np.concatenate([xe_y[sl, 0], xe_y[sl, 1]], axis=0)    # (8, L, D)
    xey = np.ascontiguousarray(ch.transpose(1, 0, 2)).reshape(L, 1, CH * D)
    return {"xey": xey, "st": st, "bias": bias_arr}


def kernel(q, a, embed, conv_w, conv_b, lin_w, lin_b):
    from concourse import bass_utils

    q = np.asarray(q); a = np.asarray(a)
    embed = np.asarray(embed, np.float32)
    conv_w = np.asarray(conv_w, np.float32)
    conv_b = np.asarray(conv_b, np.float32)
    lin_w = np.asarray(lin_w, np.float32)
    lin_b = np.asarray(lin_b, np.float32)

    # host: embedding gather + density normalization factors
    idx = np.stack([q, a], axis=1).astype(np.int64)            # (B, 2, L)
    xe = embed[idx].astype(np.float64)                         # (B, 2, L, D)
    dot = np.sum(xe * xe, axis=-1, keepdims=True) + 1e-4
    xe_y = (xe / np.sqrt(dot)).astype(np.float16)

    # host: Toeplitz band stationaries  lhsT[(l,g,dh)] = B^T,
    # B[w, w'] = W[dh, w'-w+1]  (3 diagonals)
    st = np.zeros((NL * 4 * 4, D, D), np.float16)
    for l in range(NL):
        for g in range(4):
            W = conv_w[l, g, 0, 0]                             # (4, 3)
            for dh in range(4):
                Bm = sum(W[dh, dw] * np.eye(D, k=dw - 1) for dw in range(3))
                st[(l * 4 + g) * 4 + dh] = Bm.T.astype(np.float16)
    bias_arr = np.tile(conv_b.reshape(1, -1), (D, 1)).astype(np.float32)

    if "nc" not in _CACHE:
        _CACHE["nc"] = _build_nc()
    nc = _CACHE["nc"]

    in_maps = [_prep_core_inputs(xe_y, st, bias_arr, i) for i in range(NCORES)]
    _CACHE["in_maps"] = in_maps
    res = bass_utils.run_bass_kernel_spmd(nc, in_maps, core_ids=list(range(NCORES)))

    # host: unshard + final linear + log_softmax
    q_p = np.zeros((B, D * D), np.float32)
    a_p = np.zeros((B, D * D), np.float32)
    for i in range(NCORES):
        out = res.results[i]["mp_out"]                         # (D w, CH*D)
        for s in range(CH):
            mp_T = out[:, s * D:(s + 1) * D].astype(np.float32)  # (w, j)
            flat = np.ascontiguousarray(mp_T.T).reshape(-1)    # j-major
            if s < 4:
                q_p[4 * i + s] = flat
            else:
                a_p[4 * i + s - 4] = flat
    qa = np.concatenate([q_p, a_p], axis=1)
    score = qa @ lin_w.T + lin_b
    m = score.max(axis=1, keepdims=True)
    ls = score - m
    lse = np.log(np.exp(ls).sum(axis=1, keepdims=True))
    return (ls - lse).astype(np.float32)



# revision 3
# speedup vs baseline: 3.7978x; 1.0854x over previous
"""Trainium2 Bass kernel for NnqlmCnnBasedLstm.

Math (per batch item, per input sequence q/a):
  xe = embed[idx]                      (L, D)       D = 128
  dens_t = outer(xe_t, xe_t)/(|xe_t|^2 + 1e-4)     (D, D), symmetric
  2-layer ConvLSTM over L=40 steps; each gate g:
    pre_g = conv2d([xt; h], W_g, stride=(2,1), pad=(1,1)) + b_g  on (2D, D) -> (D, D)
  c = sig(f)*c + sig(i)*tanh(cc); h = sig(o)*tanh(c)
  out = max_t h2_t  -> flatten -> concat(q,a) -> linear(2) -> log_softmax

Device strategy (8 cores, data parallel over B=32 -> 4 items/core, each with a
q-chain and an a-chain = 8 chains/core):
  * State kept TRANSPOSED: tiles are (w partitions, conv-H free).  The density
    is symmetric so layer-1 inputs need no transpose.  All matmul-path data is
    fp16 (PE runs 16-bit at 1 col/cycle vs fp32's 4) with fp32 PSUM accumulate.
  * Combined per-chain segment [P P | dens_k | h1_{k-1} | h2_{k-2} | P P]
    (388 cols) serves BOTH conv windows: layer 1 reads [dens; h1] (cols 2..257)
    and layer 2 reads [h1; h2] (cols 130..385).  h1 is written exactly once.
    Boundary fixes: layer-1 dh=3 skips j=127 and layer-2 dh=0 skips j=0 (their
    true contributions are zero-pad terms).
  * conv: out_T[w, j] = sum_{dh,dw} W[dh,dw] * inp_T[w-1+dw, 2j-1+dh].
    For each dh this is a 3-diagonal Toeplitz band matrix (over w) applied via
    the TensorEngine with a stride-2 free-axis access; 4 gates x 4 dh x 2
    halves = 32 matmuls of N<=512 per layer accumulating in PSUM.
  * Densities: ONE K=8 matmul pair per step -- lhsT = stacked xe vectors
    (8 x 128), rhs = block-diagonal (8 x 1024) holding the same vectors.
  * sigmoid/tanh (+bias) on ScalarE reading PSUM; cell updates on VectorE
    (fp16, 2x mode); gate order cs,f,i,o + split o/h halves shortens the
    cross-layer critical path.
  * Embedding gather, final linear + log_softmax on host (tiny).
"""

import os
import sys

import numpy as np

for _p in ("/opt/trn_rl_repo", "/root/.axon_site/_ro/trn_rl_repo"):
    if os.path.isdir(_p) and _p not in sys.path:
        sys.path.insert(0, _p)

B, L, D, V, NL = 32, 40, 128, 32000, 2
NCORES = 8
CH = 8            # chains per core: 4 batch items x {q, a}
SEG = 388         # [P P | dens(128) | h1(128) | h2(128) | P P]
NF = CH * SEG

_CACHE = {}


def _build_nc(L=L):
    import concourse.bass as bass
    import concourse.bacc as bacc
    import concourse.mybir as mybir
    from concourse import tile

    f32 = mybir.dt.float32
    f16 = mybir.dt.float16
    AF = mybir.ActivationFunctionType
    ALU = mybir.AluOpType

    nc = bacc.Bacc(None, target_bir_lowering=False)

    xey_d = nc.dram_tensor("xey", (L, CH, D), f16, kind="ExternalInput")
    st_d = nc.dram_tensor("st", (NL * 4 * 4, D, D), f16, kind="ExternalInput")
    bias_d = nc.dram_tensor("bias", (D, NL * 4), f32, kind="ExternalInput")
    out_d = nc.dram_tensor("mp_out", (D, CH * D), f16, kind="ExternalOutput")

    # gate order per layer: cs first (longest consumer chain), o last
    GORDER = [3, 0, 1, 2]        # reference gate index: 0=f 1=i 2=o 3=cs
    GTAG = {3: "pcs", 0: "pf", 1: "pi", 2: "po"}

    with tile.TileContext(nc) as tc:
        with (
            tc.tile_pool(name="const", bufs=1) as constp,
            tc.tile_pool(name="state", bufs=1) as statep,
            tc.tile_pool(name="gate", bufs=2) as gatep,
            tc.tile_pool(name="psum", bufs=1, space="PSUM") as psump,
        ):
            # ---- constants ----
            stT = constp.tile([D, NL * 4 * 4 * D], f16, tag="stT")
            for i in range(NL * 4 * 4):
                nc.sync.dma_start(stT[:, i * D:(i + 1) * D], st_d[i])
            bias = constp.tile([D, NL * 4], f32, tag="bias")
            nc.sync.dma_start(bias[:], bias_d[:])

            # outer-product staging (double buffered)
            xstack = [constp.tile([CH, D], f16, tag=f"xs{i}", name=f"xs{i}")
                      for i in range(2)]
            xdiag = [constp.tile([CH, CH * D], f16, tag=f"xd{i}", name=f"xd{i}")
                     for i in range(2)]
            for i in range(2):
                nc.vector.memset(xdiag[i][:], 0.0)

            # ---- persistent state ----
            bufs = [statep.tile([D, NF], f16, tag=f"b{i}", name=f"b{i}")
                    for i in range(3)]
            for bt in bufs:
                nc.vector.memset(bt[:], 0.0)
            c_l = [statep.tile([D, CH * D], f16, tag=f"c{l}", name=f"c{l}")
                   for l in range(NL)]
            for l in range(NL):
                nc.vector.memset(c_l[l][:], 0.0)
            mp = statep.tile([D, CH * D], f16, tag="mp")
            nc.vector.memset(mp[:], -60000.0)

            def seg4(t):  # (p, s, c2, two) parity view
                return t[:].rearrange("p (s c two) -> p s c two", s=CH, two=2)

            def segd(t):  # (p, s, c) view for block writes
                return t[:].rearrange("p (s c) -> p s c", s=CH)

            def dma_x(t_next):
                """Load xe vectors for step t_next into staging set t_next%2."""
                i = t_next % 2
                nc.sync.dma_start(xstack[i][:], xey_d[t_next])
                for s in range(CH):
                    nc.sync.dma_start(
                        xdiag[i][s:s + 1, s * D:(s + 1) * D], xey_d[t_next, s])

            def outer(t_next, dst):
                """Densities for step t_next -> dens block of dst buffer."""
                i = t_next % 2
                po = psump.tile([D, CH * D], f32, tag="pcs", name="po_outer")
                for half in range(2):
                    nc.tensor.matmul(
                        po[:, half * 512:(half + 1) * 512],
                        xstack[i][:],
                        xdiag[i][:, half * 512:(half + 1) * 512],
                        start=True, stop=True,
                    )
                nc.vector.tensor_copy(segd(dst)[:, :, 2:2 + D], po[:])

            dma_x(0)
            outer(0, bufs[0])
            dma_x(1)

            for t in range(L):
                cur = bufs[t % 3]          # [dens_t, h1_{t-1}, h2_{t-2}]
                nxt = bufs[(t + 1) % 3]    # gets h1_t (+ dens_{t+1} later)
                nx2 = bufs[(t + 2) % 3]    # gets h2_t

                for l in range(NL):
                    src = cur if l == 0 else nxt
                    i4 = seg4(src)
                    coff = 0 if l == 0 else 64   # window start (c pairs)

                    gt = {}
                    ps = {}
                    t1 = t2 = th = None
                    for g in GORDER:
                        pg = psump.tile([D, CH * D], f32, tag=GTAG[g],
                                        name=GTAG[g])
                        ps[g] = pg
                        pv = pg[:].rearrange("p (s j) -> p s j", s=CH)
                        for half in range(2):
                            s0 = half * 4
                            for dh in range(4):
                                idx = (l * 4 + g) * 4 + dh
                                par = (dh + 1) % 2
                                cbase = coff + (dh + 1) // 2
                                j0, j1 = 0, 128
                                if l == 0 and dh == 3:
                                    j1 = 127        # skip j=127 (pad row)
                                if l == 1 and dh == 0:
                                    j0 = 1          # skip j=0 (pad row)
                                rhs = i4[:, s0:s0 + 4,
                                         cbase + j0: cbase + j1, par]
                                nc.tensor.matmul(
                                    pv[:, s0:s0 + 4, j0:j1],
                                    stT[:, idx * D:(idx + 1) * D],
                                    rhs,
                                    start=(dh == 0), stop=(dh == 3),
                                )
                        # activation (bias folded in); o handled below in halves
                        bcol = bias[:, l * 4 + g: l * 4 + g + 1]
                        if g == 3:
                            cs = gatep.tile([D, CH * D], f16, tag="cs")
                            nc.scalar.activation(cs[:], pg[:], AF.Tanh,
                                                 bias=bcol)
                            gt[g] = cs
                        elif g != 2:
                            dst = gatep.tile([D, CH * D], f16,
                                             tag=("fg" if g == 0 else "ig"))
                            nc.scalar.activation(dst[:], pg[:], AF.Sigmoid,
                                                 bias=bcol)
                            gt[g] = dst
                        # interleave DVE cell ops as operands become ready
                        if g == 0:
                            t1 = gatep.tile([D, CH * D], f16, tag="t1")
                            nc.vector.tensor_mul(t1[:], gt[0][:], c_l[l][:])
                        if g == 1:
                            t2 = gatep.tile([D, CH * D], f16, tag="t2")
                            nc.vector.tensor_mul(t2[:], gt[1][:], gt[3][:])
                            nc.vector.tensor_add(c_l[l][:], t1[:], t2[:])
                            th = gatep.tile([D, CH * D], f16, tag="th")
                            nc.scalar.activation(th[:], c_l[l][:], AF.Tanh)

                    # o gate + h write, split by chain halves so the next
                    # layer's matmuls can start on half 1 early
                    og = gatep.tile([D, CH * D], f16, tag="og")
                    if l == 0:
                        hv = segd(nxt)[:, :, 130:130 + D]
                    else:
                        hv = segd(nx2)[:, :, 258:258 + D]
                    thv = th[:].rearrange("p (s j) -> p s j", s=CH)
                    ogv = og[:].rearrange("p (s j) -> p s j", s=CH)
                    for half in range(2):
                        s0 = half * 4
                        nc.scalar.activation(
                            og[:, s0 * D:(s0 + 4) * D],
                            ps[2][:, s0 * D:(s0 + 4) * D],
                            AF.Sigmoid, bias=bias[:, l * 4 + 2: l * 4 + 3])
                        nc.vector.tensor_mul(
                            hv[:, s0:s0 + 4, :],
                            ogv[:, s0:s0 + 4, :],
                            thv[:, s0:s0 + 4, :])

                    if l == 1:
                        nc.vector.tensor_tensor(mp[:], mp[:], hv[:, :, :],
                                                op=ALU.max)

                # prefetch next density + its staging data
                if t + 1 < L:
                    outer(t + 1, nxt)
                if t + 2 < L:
                    dma_x(t + 2)

            nc.sync.dma_start(out_d[:], mp[:])

    nc.compile()
    return nc


def _prep_core_inputs(xe_y, st, bias_arr, core):
    """xe_y: (B, 2, L, D) sqrt-normalized embeddings (axis1: 0=q, 1=a)."""
    sl = slice(4 * core, 4 * core + 4)
    # chains: s=0..3 -> q items, s=4..7 -> a items
    ch = np.concatenate([xe_y[sl, 0], xe_y[sl, 1]], axis=0)    # (8, L, D)
    xey = np.ascontiguousarray(ch.transpose(1, 0, 2))          # (L, 8, D)
    return {"xey": xey, "st": st, "bias": bias_arr}


def kernel(q, a, embed, conv_w, conv_b, lin_w, lin_b):
    from concourse import bass_utils

    q = np.asarray(q); a = np.asarray(a)
    embed = np.asarray(embed, np.float32)
    conv_w = np.asarray(conv_w, np.float32)
    conv_b = np.asarray(conv_b, np.float32)
    lin_w = np.asarray(lin_w, np.float32)
    lin_b = np.asarray(lin_b, np.float32)

    # host: embedding gather + density normalization factors
    idx = np.stack([q, a], axis=1).astype(np.int64)            # (B, 2, L)
    xe = embed[idx].astype(np.float64)                         # (B, 2, L, D)
    dot = np.sum(xe * xe, axis=-1, keepdims=True) + 1e-4
    xe_y = (xe / np.sqrt(dot)).astype(np.float16)

    # host: Toeplitz band stationaries  lhsT[(l,g,dh)] = B^T,
    # B[w, w'] = W[dh, w'-w+1]  (3 diagonals)
    st = np.zeros((NL * 4 * 4, D, D), np.float16)
    for l in range(NL):
        for g in range(4):
            W = conv_w[l, g, 0, 0]                             # (4, 3)
            for dh in range(4):
                Bm = sum(W[dh, dw] * np.eye(D, k=dw - 1) for dw in range(3))
                st[(l * 4 + g) * 4 + dh] = Bm.T.astype(np.float16)
    bias_arr = np.tile(conv_b.reshape(1, -1), (D, 1)).astype(np.float32)

    if "nc" not in _CACHE:
        _CACHE["nc"] = _build_nc()
    nc = _CACHE["nc"]

    in_maps = [_prep_core_inputs(xe_y, st, bias_arr, i) for i in range(NCORES)]
    _CACHE["in_maps"] = in_maps
    res = bass_utils.run_bass_kernel_spmd(nc, in_maps, core_ids=list(range(NCORES)))

    # host: unshard + final linear + log_softmax
    q_p = np.zeros((B, D * D), np.float32)
    a_p = np.zeros((B, D * D), np.float32)
    for i in range(NCORES):
        out = res.results[i]["mp_out"]                         # (D w, CH*D)
        for s in range(CH):
            mp_T = out[:, s * D:(s + 1) * D].astype(np.float32)  # (w, j)
            flat = np.ascontiguousarray(mp_T.T).reshape(-1)    # j-major
            if s < 4:
                q_p[4 * i + s] = flat
            else:
                a_p[4 * i + s - 4] = flat
    qa = np.concatenate([q_p, a_p], axis=1)
    score = qa @ lin_w.T + lin_b
    m = score.max(axis=1, keepdims=True)
    ls = score - m
    lse = np.log(np.exp(ls).sum(axis=1, keepdims=True))
    return (ls - lse).astype(np.float32)


# revision 8
# speedup vs baseline: 4.3960x; 1.1575x over previous
"""Trainium2 Bass kernel for NnqlmCnnBasedLstm.

Math (per batch item, per input sequence q/a):
  xe = embed[idx]                      (L, D)       D = 128
  dens_t = outer(xe_t, xe_t)/(|xe_t|^2 + 1e-4)     (D, D), symmetric
  2-layer ConvLSTM over L=40 steps; each gate g:
    pre_g = conv2d([xt; h], W_g, stride=(2,1), pad=(1,1)) + b_g  on (2D, D) -> (D, D)
  c = sig(f)*c + sig(i)*tanh(cc); h = sig(o)*tanh(c)
  out = max_t h2_t  -> flatten -> concat(q,a) -> linear(2) -> log_softmax

Device strategy (8 cores, data parallel over B=32 -> 4 items/core, each with a
q-chain and an a-chain = 8 chains/core):
  * State kept TRANSPOSED: tiles are (w partitions, conv-H free).  The density
    is symmetric so layer-1 inputs need no transpose.  All matmul-path data is
    fp16 (PE runs 16-bit at 1 col/cycle vs fp32's 4) with fp32 PSUM accumulate.
  * Combined per-chain segment [P P | dens_k | h1_{k-1} | h2_{k-2} | P P]
    (388 cols) serves BOTH conv windows: layer 1 reads [dens; h1] (cols 2..257)
    and layer 2 reads [h1; h2] (cols 130..385).  h1 is written exactly once.
    Boundary fixes: layer-1 dh=3 skips j=127 and layer-2 dh=0 skips j=0 (their
    true contributions are zero-pad terms).
  * conv: out_T[w, j] = sum_{dh,dw} W[dh,dw] * inp_T[w-1+dw, 2j-1+dh].
    For each dh this is a 3-diagonal Toeplitz band matrix (over w) applied via
    the TensorEngine with a stride-2 free-axis access; 4 gates x 4 dh x 2
    halves = 32 matmuls of N<=512 per layer accumulating in PSUM.
  * Densities: ONE K=8 matmul pair per step -- lhsT = stacked xe vectors
    (8 x 128), rhs = block-diagonal (8 x 1024) holding the same vectors.
  * sigmoid/tanh (+bias) on ScalarE reading PSUM; cell updates on VectorE
    (fp16, 2x mode); gate order cs,f,i,o + split o/h halves shortens the
    cross-layer critical path.
  * Embedding gather, final linear + log_softmax on host (tiny).
"""

import os
import sys

import numpy as np

for _p in ("/opt/trn_rl_repo", "/root/.axon_site/_ro/trn_rl_repo"):
    if os.path.isdir(_p) and _p not in sys.path:
        sys.path.insert(0, _p)

B, L, D, V, NL = 32, 40, 128, 32000, 2
NCORES = 8
CH = 8            # chains per core: 4 batch items x {q, a}
SEG = 388         # [P P | dens(128) | h1(128) | h2(128) | P P]
NF = CH * SEG

_CACHE = {}


def _build_nc(L=L):
    import concourse.bass as bass
    import concourse.bacc as bacc
    import concourse.mybir as mybir
    from concourse import tile

    f32 = mybir.dt.float32
    f16 = mybir.dt.float16
    AF = mybir.ActivationFunctionType
    ALU = mybir.AluOpType

    nc = bacc.Bacc(None, target_bir_lowering=False)

    xey_d = nc.dram_tensor("xey", (L, CH, D), f16, kind="ExternalInput")
    st_d = nc.dram_tensor("st", (NL * 4 * 4, D, D), f16, kind="ExternalInput")
    bias_d = nc.dram_tensor("bias", (D, NL * 4), f32, kind="ExternalInput")
    out_d = nc.dram_tensor("mp_out", (D, CH * D), f16, kind="ExternalOutput")

    # gate order per layer: cs first (longest consumer chain), o last
    GORDER = [3, 0, 1, 2]        # reference gate index: 0=f 1=i 2=o 3=cs
    GTAG = {3: "pcs", 0: "pf", 1: "pi", 2: "po"}

    with tile.TileContext(nc) as tc:
        with (
            tc.tile_pool(name="const", bufs=1) as constp,
            tc.tile_pool(name="state", bufs=1) as statep,
            tc.tile_pool(name="gate", bufs=2) as gatep,
            tc.tile_pool(name="psum", bufs=1, space="PSUM") as psump,
        ):
            # ---- constants ----
            stT = constp.tile([D, NL * 4 * 4 * D], f16, tag="stT")
            for i in range(NL * 4 * 4):
                nc.sync.dma_start(stT[:, i * D:(i + 1) * D], st_d[i])
            bias = constp.tile([D, NL * 4], f32, tag="bias")
            nc.sync.dma_start(bias[:], bias_d[:])

            # outer-product staging (double buffered)
            xstack = [constp.tile([CH, D], f16, tag=f"xs{i}", name=f"xs{i}")
                      for i in range(2)]
            xdiag = [constp.tile([CH, CH * D], f16, tag=f"xd{i}", name=f"xd{i}")
                     for i in range(2)]
            for i in range(2):
                nc.vector.memset(xdiag[i][:], 0.0)

            # ---- persistent state ----
            bufs = [statep.tile([D, NF], f16, tag=f"b{i}", name=f"b{i}")
                    for i in range(3)]
            for bt in bufs:
                nc.vector.memset(bt[:], 0.0)
            c_l = [statep.tile([D, CH * D], f16, tag=f"c{l}", name=f"c{l}")
                   for l in range(NL)]
            for l in range(NL):
                nc.vector.memset(c_l[l][:], 0.0)
            mp = statep.tile([D, CH * D], f16, tag="mp")
            nc.vector.memset(mp[:], -60000.0)

            def seg4(t):  # (p, s, c2, two) parity view
                return t[:].rearrange("p (s c two) -> p s c two", s=CH, two=2)

            def segd(t):  # (p, s, c) view for block writes
                return t[:].rearrange("p (s c) -> p s c", s=CH)

            def dma_x(t_next):
                """Load xe vectors for step t_next into staging set t_next%2."""
                i = t_next % 2
                nc.sync.dma_start(xstack[i][:], xey_d[t_next])
                for s in range(CH):
                    nc.sync.dma_start(
                        xdiag[i][s:s + 1, s * D:(s + 1) * D], xey_d[t_next, s])

            def outer(t_next, dst):
                """Densities for step t_next -> dens block of dst buffer."""
                i = t_next % 2
                po = psump.tile([D, CH * D], f32, tag="pf", name="po_outer")
                for half in range(2):
                    nc.tensor.matmul(
                        po[:, half * 512:(half + 1) * 512],
                        xstack[i][:],
                        xdiag[i][:, half * 512:(half + 1) * 512],
                        start=True, stop=True,
                    )
                nc.vector.tensor_copy(segd(dst)[:, :, 2:2 + D], po[:])

            dma_x(0)
            outer(0, bufs[0])
            dma_x(1)

            for t in range(L):
                cur = bufs[t % 3]          # [dens_t, h1_{t-1}, h2_{t-2}]
                nxt = bufs[(t + 1) % 3]    # gets h1_t (+ dens_{t+1} later)
                nx2 = bufs[(t + 2) % 3]    # gets h2_t

                for l in range(NL):
                    src = cur if l == 0 else nxt
                    i4 = seg4(src)
                    coff = 0 if l == 0 else 64   # window start (c pairs)

                    def mm_gate(g, pv):
                        for half in range(2):
                            s0 = half * 4
                            for dh in range(4):
                                idx = (l * 4 + g) * 4 + dh
                                par = (dh + 1) % 2
                                cbase = coff + (dh + 1) // 2
                                j0, j1 = 0, 128
                                if l == 0 and dh == 3:
                                    j1 = 127        # skip j=127 (pad row)
                                if l == 1 and dh == 0:
                                    j0 = 1          # skip j=0 (pad row)
                                rhs = i4[:, s0:s0 + 4,
                                         cbase + j0: cbase + j1, par]
                                nc.tensor.matmul(
                                    pv[:, s0:s0 + 4, j0:j1],
                                    stT[:, idx * D:(idx + 1) * D],
                                    rhs,
                                    start=(dh == 0), stop=(dh == 3),
                                )

                    H = 512  # half width in flat columns
                    ps = {}
                    gt = {}
                    # matmuls + gate ACTs (cs, f, i) split by chain halves
                    for g in (3, 0, 1):
                        pg = psump.tile([D, CH * D], f32, tag=GTAG[g],
                                        name=GTAG[g])
                        ps[g] = pg
                        mm_gate(g, pg[:].rearrange("p (s j) -> p s j", s=CH))
                        dst = gatep.tile([D, CH * D], f16,
                                         tag={3: "cs", 0: "fg", 1: "ig"}[g])
                        gt[g] = dst
                        bcol = bias[:, l * 4 + g: l * 4 + g + 1]
                        fn = AF.Tanh if g == 3 else AF.Sigmoid
                        for h in range(2):
                            nc.scalar.activation(dst[:, h * H:(h + 1) * H],
                                                 pg[:, h * H:(h + 1) * H],
                                                 fn, bias=bcol)
                        # cell ops as operands appear
                        if g == 0:
                            t1 = gatep.tile([D, CH * D], f16, tag="t1")
                            for h in range(2):
                                nc.vector.tensor_mul(
                                    t1[:, h * H:(h + 1) * H],
                                    gt[0][:, h * H:(h + 1) * H],
                                    c_l[l][:, h * H:(h + 1) * H])

                    # o-gate matmuls
                    pg = psump.tile([D, CH * D], f32, tag=GTAG[2], name=GTAG[2])
                    ps[2] = pg
                    mm_gate(2, pg[:].rearrange("p (s j) -> p s j", s=CH))

                    # half-pipelined cell tail: t2, c, th, og, h per half
                    t2 = gatep.tile([D, CH * D], f16, tag="t2")
                    th = gatep.tile([D, CH * D], f16, tag="th")
                    og = gatep.tile([D, CH * D], f16, tag="og")
                    if l == 0:
                        hv = segd(nxt)[:, :, 130:130 + D]
                    else:
                        hv = segd(nx2)[:, :, 258:258 + D]
                    thv = th[:].rearrange("p (s j) -> p s j", s=CH)
                    ogv = og[:].rearrange("p (s j) -> p s j", s=CH)
                    for h in range(2):
                        sl = slice(h * H, (h + 1) * H)
                        s0 = h * 4
                        nc.vector.tensor_mul(t2[:, sl], gt[1][:, sl],
                                             gt[3][:, sl])
                        nc.vector.tensor_add(c_l[l][:, sl], t1[:, sl],
                                             t2[:, sl])
                        nc.scalar.activation(th[:, sl], c_l[l][:, sl], AF.Tanh)
                        nc.scalar.activation(
                            og[:, sl], ps[2][:, sl], AF.Sigmoid,
                            bias=bias[:, l * 4 + 2: l * 4 + 3])
                        nc.vector.tensor_mul(hv[:, s0:s0 + 4, :],
                                             ogv[:, s0:s0 + 4, :],
                                             thv[:, s0:s0 + 4, :])
                        # after layer 2's first h-half: slot the next step's
                        # outer product right behind the o-matmuls on the PE
                        # queue ("pf" bank is long drained) and its dens CAST
                        # ahead of non-critical vector work
                        if l == 1 and h == 0 and t + 1 < L:
                            outer(t + 1, nxt)

                    if l == 1:
                        nc.vector.tensor_tensor(mp[:], mp[:], hv[:, :, :],
                                                op=ALU.max)

                # prefetch staging data for the step after next
                if t + 2 < L:
                    dma_x(t + 2)

            nc.sync.dma_start(out_d[:], mp[:])

    nc.compile()
    return nc


def _prep_core_inputs(xe_y, st, bias_arr, core):
    """xe_y: (B, 2, L, D) sqrt-normalized embeddings (axis1: 0=q, 1=a)."""
    sl = slice(4 * core, 4 * core + 4)
    # chains: s=0..3 -> q items, s=4..7 -> a items
    ch = np.concatenate([xe_y[sl, 0], xe_y[sl, 1]], axis=0)    # (8, L, D)
    xey = np.ascontiguousarray(ch.transpose(1, 0, 2))          # (L, 8, D)
    return {"xey": xey, "st": st, "bias": bias_arr}


def kernel(q, a, embed, conv_w, conv_b, lin_w, lin_b):
    from concourse import bass_utils

    q = np.asarray(q); a = np.asarray(a)
    embed = np.asarray(embed, np.float32)
    conv_w = np.asarray(conv_w, np.float32)
    conv_b = np.asarray(conv_b, np.float32)
    lin_w = np.asarray(lin_w, np.float32)
    lin_b = np.asarray(lin_b, np.float32)

    # host: embedding gather + density normalization factors
    idx = np.stack([q, a], axis=1).astype(np.int64)            # (B, 2, L)
    xe = embed[idx].astype(np.float64)                         # (B, 2, L, D)
    dot = np.sum(xe * xe, axis=-1, keepdims=True) + 1e-4
    xe_y = (xe / np.sqrt(dot)).astype(np.float16)

    # host: Toeplitz band stationaries  lhsT[(l,g,dh)] = B^T,
    # B[w, w'] = W[dh, w'-w+1]  (3 diagonals)
    st = np.zeros((NL * 4 * 4, D, D), np.float16)
    for l in range(NL):
        for g in range(4):
            W = conv_w[l, g, 0, 0]                             # (4, 3)
            for dh in range(4):
                Bm = sum(W[dh, dw] * np.eye(D, k=dw - 1) for dw in range(3))
                st[(l * 4 + g) * 4 + dh] = Bm.T.astype(np.float16)
    bias_arr = np.tile(conv_b.reshape(1, -1), (D, 1)).astype(np.float32)

    if "nc" not in _CACHE:
        _CACHE["nc"] = _build_nc()
    nc = _CACHE["nc"]

    in_maps = [_prep_core_inputs(xe_y, st, bias_arr, i) for i in range(NCORES)]
    _CACHE["in_maps"] = in_maps
    res = bass_utils.run_bass_kernel_spmd(nc, in_maps, core_ids=list(range(NCORES)))

    # host: unshard + final linear + log_softmax
    q_p = np.zeros((B, D * D), np.float32)
    a_p = np.zeros((B, D * D), np.float32)
    for i in range(NCORES):
        out = res.results[i]["mp_out"]                         # (D w, CH*D)
        for s in range(CH):
            mp_T = out[:, s * D:(s + 1) * D].astype(np.float32)  # (w, j)
            flat = np.ascontiguousarray(mp_T.T).reshape(-1)    # j-major
            if s < 4:
                q_p[4 * i + s] = flat
            else:
                a_p[4 * i + s - 4] = flat
    qa = np.concatenate([q_p, a_p], axis=1)
    score = qa @ lin_w.T + lin_b
    m = score.max(axis=1, keepdims=True)
    ls = score - m
    lse = np.log(np.exp(ls).sum(axis=1, keepdims=True))
    return (ls - lse).astype(np.float32)
